# revision 4
# baseline (speedup 1.0000x reference)
"""Multi-head attention (B=2, F=T=2048, 16 heads x 64) on 8 TRN2 NeuronCores.

Sharding: core c = (batch b = c//4) x (head-group g = c%4, 4 heads each).

V6: software-pipelined reps. Each attention body projects q/k m-tile 1 early
(consumed by its own heads 2/3), and q/k m-tile 0 + all of v late (consumed by
the NEXT body's heads 0/1 / ctx). A one-time prologue seeds rep 0. The v tiles
ping-pong across a 2-body unroll inside For_i, so in the steady reps loop the
next body's first scores depend on nothing newer than mid-previous-body: the
inter-rep ACT gap collapses to the pipeline rings.

Other structure (from V2..V5):
- ctx natural layout (em stationary, v+ones moving, N=65), chunks packed 7 per
  psum bank (matmul out must stay in one 2KB bank; accumulation groups are
  bank-granular: one start / one stop per bank).
- Softmax denominator = 65th v column; normalize = DVE reciprocal +
  per-partition tensor_scalar multiply.
- exp writes the em tile, mask multiply runs in place (bf16 2x DVE).
- q/k biases fused into psum->sbuf copies; v bias via K=1 ones-row matmul.
- ctx trails scores/exp by CTX_DELAY iters; normalizes spread over following
  iters; per-head output DMA in natural [F, 256] bf16.
"""
import sys
import numpy as np

for _p in ("/opt/trn_rl_repo",):
    if _p not in sys.path:
        sys.path.insert(0, _p)

import ml_dtypes

bf16 = ml_dtypes.bfloat16

N_HEADS_TOTAL = 16
HEAD_DIM = 64
HIDDEN = N_HEADS_TOTAL * HEAD_DIM
N_CORES = 8
NH = 4  # heads per core


def build_nc(F=2048, T=2048, D=1024, reps=1):
    import os
    from contextlib import nullcontext
    from concourse import bass, bacc, tile, mybir

    KN = lambda name, d: int(os.environ.get('K_' + name, d))

    f32 = mybir.dt.float32
    b16 = mybir.dt.bfloat16
    KT = D // 128
    TT = T // 128
    MT = (NH * HEAD_DIM) // 128
    DG = NH * HEAD_DIM
    HF = F // 2
    NCH = F // 128
    VW = HEAD_DIM + 1
    assert reps == 1 or reps % 2 == 0, "reps must be 1 or even"

    nc = bacc.Bacc(None, target_bir_lowering=False, debug=False)

    xt_d = nc.declare_dram_parameter("xt", [D, F], b16, isOutput=False)
    yt_d = nc.declare_dram_parameter("yt", [D, T], b16, isOutput=False)
    mk_d = nc.declare_dram_parameter("maskT", [T, F], b16, isOutput=False)
    wq_d = nc.declare_dram_parameter("wq", [D, DG], b16, isOutput=False)
    wk_d = nc.declare_dram_parameter("wk", [D, DG], b16, isOutput=False)
    wv_d = nc.declare_dram_parameter("wv", [D, DG], b16, isOutput=False)
    bq_d = nc.declare_dram_parameter("bq", [128, MT], f32, isOutput=False)
    bk_d = nc.declare_dram_parameter("bk", [128, MT], f32, isOutput=False)
    bvr_d = nc.declare_dram_parameter("bvr", [1, DG], b16, isOutput=False)
    out_d = nc.declare_dram_parameter("out", [F, DG], b16, isOutput=True)

    EXPF = mybir.ActivationFunctionType.Exp
    CTX_DELAY = KN("CTX_DELAY", 3)
    NORM_RATE = KN("NORM_RATE", 6)
    HOLD = KN("HOLD", 3)

    with tile.TileContext(nc) as tc:
        with (
            tc.tile_pool(name="res", bufs=1) as res,
            tc.tile_pool(name="npool", bufs=2) as npool,
            tc.tile_pool(name="empool", bufs=KN("EMPOOL", 11)) as empool,
            tc.tile_pool(name="spool", bufs=2, space="PSUM") as spool,
            tc.tile_pool(name="ppool", bufs=1, space="PSUM") as ppool,
            tc.tile_pool(name="cpool", bufs=1, space="PSUM") as cpool,
        ):
            # ---- resident SBUF ----
            xt_sb = res.tile([128, KT, F], b16, tag="xt")
            yt_sb = res.tile([128, KT, T], b16, tag="yt")
            mask_sb = res.tile([128, TT, F], b16, tag="mask")
            wq_sb = res.tile([128, KT, DG], b16, tag="wq")
            wk_sb = res.tile([128, KT, DG], b16, tag="wk")
            wv_sb = res.tile([128, KT, DG], b16, tag="wv")
            bq_sb = res.tile([128, MT], f32, tag="bq")
            bk_sb = res.tile([128, MT], f32, tag="bk")
            bvr_sb = res.tile([1, DG], b16, tag="bvr")
            ones = res.tile([1, 128], b16, tag="ones")
            qT_sb = res.tile([128, MT, F], b16, tag="qT")
            kT_sb = res.tile([128, MT, T], b16, tag="kT")
            v0_sb = res.tile([128, TT, NH, VW], b16, tag="v0")
            v1_sb = res.tile([128, TT, NH, VW], b16, tag="v1")
            out_sb = res.tile([128, NCH, DG], b16, tag="out")

            nc.vector.memset(ones[:], 1.0)
            nc.vector.memset(v0_sb[:, :, :, HEAD_DIM], 1.0)
            nc.vector.memset(v1_sb[:, :, :, HEAD_DIM], 1.0)

            def input_dmas(mask=True):
                # mask tiles issue from the ACT engine's HWDGE queue: their
                # WARs (prev body's h3 reads) release earliest, so the mask
                # stream flows during the previous body's tail without
                # blocking the SP queue where xt/yt re-load for projections
                if mask:
                    for t in range(TT):
                        nc.sync.dma_start(mask_sb[:, t, :],
                                          mk_d[t * 128:(t + 1) * 128, :])
                nc.sync.dma_start(wq_sb[:],
                                  wq_d.rearrange("(k p) n -> p k n", p=128))
                nc.sync.dma_start(wk_sb[:],
                                  wk_d.rearrange("(k p) n -> p k n", p=128))
                nc.sync.dma_start(wv_sb[:],
                                  wv_d.rearrange("(k p) n -> p k n", p=128))
                nc.sync.dma_start(bq_sb[:], bq_d[:])
                nc.sync.dma_start(bk_sb[:], bk_d[:])
                nc.sync.dma_start(bvr_sb[:], bvr_d[:])
                for k in range(KT):
                    nc.sync.dma_start(xt_sb[:, k, :],
                                      xt_d[k * 128:(k + 1) * 128, :])
                    nc.sync.dma_start(yt_sb[:, k, :],
                                      yt_d[k * 128:(k + 1) * 128, :])

            def qk_block(dst, w_sb, b_sb, act_sb, m, c0, alt=False):
                # [128, 512] proj psum block; alt=True borrows the ctx pool's
                # banks (prologue only, before any ctx tile exists)
                if alt:
                    ps = cpool.tile([128, 512], f32, tag="ctx", name="qkalt")
                else:
                    ps = ppool.tile([128, 512], f32, tag="p")
                for k in range(KT):
                    nc.tensor.matmul(
                        ps[:], w_sb[:, k, m * 128:(m + 1) * 128],
                        act_sb[:, k, c0:c0 + 512],
                        start=(k == 0), stop=(k == KT - 1))
                nc.vector.tensor_scalar_add(dst[:, m, c0:c0 + 512], ps[:],
                                            b_sb[:, m:m + 1])

            def v_block(t, vdst):
                ps = ppool.tile([128, DG], f32, tag="p")
                for k in range(KT):
                    nc.tensor.matmul(
                        ps[:], yt_sb[:, k, t * 128:(t + 1) * 128], wv_sb[:, k, :],
                        start=(k == 0), stop=False)
                nc.tensor.matmul(ps[:], ones[0:1, :], bvr_sb[0:1, :],
                                 start=False, stop=True)
                nc.vector.tensor_copy(
                    vdst[:, t, :, 0:HEAD_DIM],
                    ps.rearrange("p (h d) -> p h d", h=NH))

            CPB = 7  # ctx chunks per psum bank

            def chunk_ap(ctx_ps, c, w=VW):
                b, j = c // CPB, c % CPB
                return ctx_ps[:, b, j * VW:j * VW + w]

            def attention_body(vc, vn):
                """One full attention pass reading q/k m0 + vc from the
                previous body (or prologue); projects m1 for its own heads
                2/3 and m0 + vn for the next body."""
                input_dmas()

                # filler schedule: iter -> list of emitters (all -> ppool)
                sched = {}

                def add(i, fn):
                    sched.setdefault(i, []).append(fn)

                # Uniform windowed schedule, kept under the ACT rate
                # everywhere (PE debt starves ACT through the 2-slot scores
                # ring). m1 early (own heads 2/3, xt/yt land ~iter 9);
                # v spread mid/late; m0 late (next body's heads 0/1).
                for j, c0 in enumerate(range(0, F, 512)):
                    add(16 + 4 * j, lambda c0=c0: qk_block(
                        qT_sb, wq_sb, bq_sb, xt_sb, 1, c0))
                    add(18 + 4 * j, lambda c0=c0: qk_block(
                        kT_sb, wk_sb, bk_sb, yt_sb, 1, c0))
                for j in range(16):
                    add(32 + 2 * j, lambda t=j: v_block(t, vn))
                for j, c0 in enumerate(range(0, F, 512)):
                    add(43 + 4 * j, lambda c0=c0: qk_block(
                        qT_sb, wq_sb, bq_sb, xt_sb, 0, c0))
                    add(45 + 4 * j, lambda c0=c0: qk_block(
                        kT_sb, wk_sb, bk_sb, yt_sb, 0, c0))

                ctx_tiles = {}
                pend = []
                norm_pend = []
                ctx_hold = [0]

                def ctx_mms(hh, td, ems):
                    if hh not in ctx_tiles:
                        ctx_tiles[hh] = cpool.tile([128, 3, 512], f32,
                                                   tag="ctx", name=f"ctx{hh}")
                    ctx_ps = ctx_tiles[hh]
                    for c in range(NCH):
                        em = ems[c // (NCH // 2)]
                        cc = c % (NCH // 2)
                        nc.tensor.matmul(
                            chunk_ap(ctx_ps, c),
                            em[:, cc * 128:(cc + 1) * 128],
                            vc[:, td, hh, :],
                            start=(td == 0 and c % CPB == 0),
                            stop=(td == TT - 1 and c in (6, 13, 15)))
                    if td == TT - 1:
                        r_sb = npool.tile([128, NCH], f32, tag="r")
                        for b in range(3):
                            n = min(CPB, NCH - b * CPB)
                            den = ctx_ps[:, b, 0:n * VW].rearrange(
                                "p (c w) -> p c w", w=VW)[:, :, HEAD_DIM]
                            nc.vector.reciprocal(r_sb[:, b * CPB:b * CPB + n],
                                                 den)

                        def norm_c(c, hh=hh, r_sb=r_sb, ctx_ps=ctx_ps):
                            nc.vector.tensor_scalar_mul(
                                out_sb[:, c, hh * HEAD_DIM:(hh + 1) * HEAD_DIM],
                                chunk_ap(ctx_ps, c, HEAD_DIM), r_sb[:, c:c + 1])
                        norm_pend.extend(
                            lambda c=c, f=norm_c: f(c) for c in range(NCH))
                        norm_pend.append(lambda hh=hh: nc.sync.dma_start(
                            out_d[:, hh * HEAD_DIM:(hh + 1) * HEAD_DIM]
                            .rearrange("(c p) n -> p c n", p=128),
                            out_sb[:, :, hh * HEAD_DIM:(hh + 1) * HEAD_DIM]))

                for h in range(NH):
                    hp = (h % 2) * 64
                    hm = h // 2
                    for t in range(TT):
                        it = h * TT + t
                        kh = kT_sb[hp:hp + 64, hm, t * 128:(t + 1) * 128]
                        ems = []
                        for half in range(2):
                            h0 = half * HF
                            s_ps = spool.tile([128, HF], f32, tag="s")
                            for cs in range(0, HF, 512):
                                nc.tensor.matmul(
                                    s_ps[:, cs:cs + 512], kh,
                                    qT_sb[hp:hp + 64, hm,
                                          h0 + cs:h0 + cs + 512],
                                    start=True, stop=True)
                            em_sb = empool.tile([128, HF], b16, tag="em")
                            nc.scalar.activation(em_sb[:], s_ps[:], EXPF,
                                                 scale=0.125)
                            nc.vector.tensor_mul(em_sb[:], em_sb[:],
                                                 mask_sb[:, t, h0:h0 + HF])
                            ems.append(em_sb)
                        for fn in sched.pop(it, ()):
                            fn()
                        if ctx_hold[0] > 0:
                            ctx_hold[0] -= 1
                        else:
                            emitted = 0
                            while pend and len(pend) >= CTX_DELAY and emitted < 2:
                                entry = pend.pop(0)
                                ctx_mms(*entry)
                                emitted += 1
                                if entry[1] == TT - 1:
                                    ctx_hold[0] = HOLD
                                    break
                        for _ in range(NORM_RATE):
                            if norm_pend:
                                norm_pend.pop(0)()
                        pend.append((h, t, ems))
                # leftover fillers (shouldn't exist), ctx tail, norms
                for i in sorted(sched):
                    for fn in sched[i]:
                        fn()
                while pend:
                    ctx_mms(*pend.pop(0))
                while norm_pend:
                    norm_pend.pop(0)()

            # ---- prologue: seed q/k m0 and v0 for the first body ----
            input_dmas(mask=False)
            for j, c0 in enumerate(range(0, F, 512)):
                qk_block(qT_sb, wq_sb, bq_sb, xt_sb, 0, c0, alt=(j % 2 == 1))
                qk_block(kT_sb, wk_sb, bk_sb, yt_sb, 0, c0, alt=(j % 2 == 0))
            for t in range(TT):
                v_block(t, v0_sb)

            if reps == 1:
                attention_body(v0_sb, v1_sb)
            else:
                with tc.For_i(0, reps // 2, 1):
                    attention_body(v0_sb, v1_sb)
                    attention_body(v1_sb, v0_sb)

    return nc


_CACHE = {}
TRACE = False


def _get_nc():
    if "nc" not in _CACHE:
        nc = build_nc()
        nc.compile()
        _CACHE["nc"] = nc
    return _CACHE["nc"]


def prep_in_maps(from_tensor, to_tensor, attention_mask, Wq, bq, Wk, bk, Wv, bv):
    from_tensor = np.asarray(from_tensor, np.float32)
    to_tensor = np.asarray(to_tensor, np.float32)
    attention_mask = np.asarray(attention_mask)
    in_maps = []
    for c in range(N_CORES):
        b, g = c // 4, c % 4
        sl = slice(g * 256, (g + 1) * 256)
        bqs = np.asarray(bq, np.float32)[sl]
        bks = np.asarray(bk, np.float32)[sl]
        bvs = np.asarray(bv, np.float32)[sl]
        in_maps.append({
            "xt": np.ascontiguousarray(from_tensor[b].T).astype(bf16),
            "yt": np.ascontiguousarray(to_tensor[b].T).astype(bf16),
            "maskT": np.ascontiguousarray(
                attention_mask[b].T.astype(np.float32)).astype(bf16),
            "wq": np.ascontiguousarray(np.asarray(Wq, np.float32)[:, sl]).astype(bf16),
            "wk": np.ascontiguousarray(np.asarray(Wk, np.float32)[:, sl]).astype(bf16),
            "wv": np.ascontiguousarray(np.asarray(Wv, np.float32)[:, sl]).astype(bf16),
            "bq": np.ascontiguousarray(bqs.reshape(2, 128).T),
            "bk": np.ascontiguousarray(bks.reshape(2, 128).T),
            "bvr": np.ascontiguousarray(bvs[None, :]).astype(bf16),
        })
    return in_maps


def gather_out(per_core_outs, B, F):
    out = np.zeros((B, F, HIDDEN), np.float32)
    for c in range(N_CORES):
        b, g = c // 4, c % 4
        out[b, :, g * 256:(g + 1) * 256] = np.asarray(per_core_outs[c],
                                                      dtype=np.float32)
    return out


def kernel(from_tensor, to_tensor, attention_mask, Wq, bq, Wk, bk, Wv, bv):
    from concourse.bass_utils import run_bass_kernel_spmd

    B, F, _ = np.asarray(from_tensor).shape
    nc = _get_nc()
    in_maps = prep_in_maps(from_tensor, to_tensor, attention_mask,
                           Wq, bq, Wk, bk, Wv, bv)
    res = run_bass_kernel_spmd(nc, in_maps, core_ids=list(range(N_CORES)),
                               trace=TRACE)
    _CACHE["last_result"] = res
    return gather_out([res.results[c]["out"] for c in range(N_CORES)], B, F)
